# revision 17
# baseline (speedup 1.0000x reference)
"""Trainium2 Bass kernel for the L2Prompt retrieval-KNN module.

Data-parallel over batch: each of the 8 NeuronCores processes 1024 of the
8192 ppg rows; the 1024x1024 key/prompt pools and conv params are replicated.

Per-core pipeline (all shapes [partition, free]):
  prep:  pcnn = relu(conv1d_k3(prompt_pool) + b)           [P=1024, D=1024]
         kn   = keys / max(||keys||, eps), KT = kn^T        (PE transpose)
         pn   = x / max(||x||, eps),       XT = pn^T        (PE transpose)
  per 128-row batch tile:
         cos  = pn @ kn^T                                   (PE matmul, PSUM)
         e    = exp(1 - cos), S1 = sum(e)                   (ACT, fused accum)
         S3   = sum(cos * e)                                (DVE fused reduce)
         m8   = top-8 cos per row                           (DVE max8)
         mr   = cos with top-8 replaced by 2.0              (DVE match_replace)
         mask = relu(2*mr - 3)  -> one-hot of top-8         (ACT)
         maskT via PE transpose; psum = mask @ pcnn         (PE matmul)
         minmax-normalize psum rows, add x, DMA out
  scalars: entropy_row = ln(S1) + S3/S1 (host subtracts 1), score via sum(m8);
         per-core partition reduction by matmul with ones; host combines.
"""

import os
import numpy as np

STAGE = int(os.environ.get("KSTAGE", "5"))
BZ, P, D, K, NCORES = 8192, 1024, 1024, 8, 8
BL = BZ // NCORES          # rows per core
NT = BL // 128             # 128-row tiles per core
ND = D // 128              # 128-wide d-chunks
EPS = 1e-8
USE_F32R = True            # float32r (1 cyc/row) for the gather matmul only:
                           # top-8 selection needs exact fp32 cos, but the
                           # mask@pcnn product only sees ~1.5e-4 rounding.

_built = {}


def _build():
    if "nc" in _built:
        return _built["nc"]

    import concourse.bacc as bacc
    import concourse.tile as tile
    from concourse import mybir

    F32 = mybir.dt.float32
    F32G = mybir.dt.float32r if USE_F32R else F32  # gather-matmul operand dtype
    AL = mybir.AluOpType
    ACTF = mybir.ActivationFunctionType
    AX = mybir.AxisListType

    nc = bacc.Bacc("TRN2", target_bir_lowering=False, debug=False)

    ppg_d = nc.dram_tensor("ppg", [BL, D], F32, kind="ExternalInput").ap()
    keys_d = nc.dram_tensor("keys", [P, D], F32, kind="ExternalInput").ap()
    pp_d = nc.dram_tensor("pp", [P, D], F32, kind="ExternalInput").ap()
    w_d = nc.dram_tensor("cw", [128, 3], F32, kind="ExternalInput").ap()
    b_d = nc.dram_tensor("cb", [128, 1], F32, kind="ExternalInput").ap()
    id_d = nc.dram_tensor("ident", [128, 128], F32, kind="ExternalInput").ap()
    out_d = nc.dram_tensor("out", [BL, D], F32, kind="ExternalOutput").ap()
    part_d = nc.dram_tensor("partials", [128, 2], F32, kind="ExternalOutput").ap()

    with tile.TileContext(nc) as tc:
        with (
            tc.tile_pool(name="big", bufs=1) as big,
            tc.tile_pool(name="prep", bufs=3) as prep,
            tc.tile_pool(name="work", bufs=2) as work,
            tc.tile_pool(name="small", bufs=2) as small,
            tc.tile_pool(name="psA", bufs=1, space="PSUM") as psA,
            tc.tile_pool(name="psC", bufs=1, space="PSUM") as psC,
            tc.tile_pool(name="psG", bufs=2, space="PSUM") as psG,
        ):
            ident = big.tile([128, 128], F32)
            nc.sync.dma_start(ident[:], id_d[:])
            acc_t = big.tile([128, 2], F32)
            nc.gpsimd.memset(acc_t[:], 0.0)
            neg3 = big.tile([128, 1], F32)
            nc.gpsimd.memset(neg3[:], -3.0)

            # conv params (pre-broadcast on host)
            w_bc = big.tile([128, 3], F32)
            nc.sync.dma_start(w_bc[:], w_d[:])
            b_bc = big.tile([128, 1], F32)
            nc.sync.dma_start(b_bc[:], b_d[:])

            X = big.tile([128, NT, D], F32)     # ppg rows, natural layout
            for j in range(NT):
                nc.sync.dma_start(X[:, j, :], ppg_d[j * 128:(j + 1) * 128, :])
            XT = big.tile([128, ND, BL], F32)   # pn transposed: [d, (chunk, b)]
            KT = big.tile([128, ND, P], F32)    # kn transposed
            PC = big.tile([128, NT, D], F32G)   # pcnn (prompt pool thru conv)

            # ---- keys: normalize rows, transpose ----
            for j in range(NT):
                kraw = prep.tile([128, D], F32, tag="prep")
                nc.sync.dma_start(kraw[:], keys_d[j * 128:(j + 1) * 128, :])
                ssq = small.tile([128, 1], F32, tag="ssq")
                scr = work.tile([128, D], F32, tag="escr")
                nc.scalar.activation(scr[:], kraw[:], ACTF.Square, accum_out=ssq[:])
                nrm = small.tile([128, 1], F32, tag="nrm")
                nc.scalar.sqrt(nrm[:], ssq[:])
                nc.vector.tensor_scalar_max(nrm[:], nrm[:], EPS)
                inv = small.tile([128, 1], F32, tag="inv")
                nc.vector.reciprocal(inv[:], nrm[:])
                knt = prep.tile([128, D], F32, tag="prep")
                nc.scalar.mul(knt[:], kraw[:], inv[:])
                tp = psA.tile([128, ND, 128], F32, tag="tp")
                for i in range(ND):
                    nc.tensor.transpose(tp[:, i, :], knt[:, i * 128:(i + 1) * 128], ident[:])
                nc.scalar.copy(KT[:, :, j * 128:(j + 1) * 128], tp[:])

            # ---- ppg: normalize rows, transpose ----
            for j in range(NT):
                xj = X[:, j, :]
                ssq = small.tile([128, 1], F32, tag="ssq")
                scr = work.tile([128, D], F32, tag="escr")
                nc.scalar.activation(scr[:], xj, ACTF.Square, accum_out=ssq[:])
                nrm = small.tile([128, 1], F32, tag="nrm")
                nc.scalar.sqrt(nrm[:], ssq[:])
                nc.vector.tensor_scalar_max(nrm[:], nrm[:], EPS)
                inv = small.tile([128, 1], F32, tag="inv")
                nc.vector.reciprocal(inv[:], nrm[:])
                pn = prep.tile([128, D], F32, tag="prep")
                nc.scalar.mul(pn[:], xj, inv[:])
                tp = psA.tile([128, ND, 128], F32, tag="tp")
                for i in range(ND):
                    nc.tensor.transpose(tp[:, i, :], pn[:, i * 128:(i + 1) * 128], ident[:])
                nc.scalar.copy(XT[:, :, j * 128:(j + 1) * 128], tp[:])

            # ---- prompt pool: conv1d(k=3, pad=1) + bias, relu ----
            for j in range(NT):
                praw = prep.tile([128, D], F32, tag="prep")
                nc.sync.dma_start(praw[:], pp_d[j * 128:(j + 1) * 128, :])
                cv = work.tile([128, D], F32, tag="cv")
                nc.vector.tensor_scalar_mul(cv[:], praw[:], w_bc[:, 1:2])
                nc.vector.scalar_tensor_tensor(
                    out=cv[:, 1:D], in0=praw[:, 0:D - 1], scalar=w_bc[:, 0:1],
                    in1=cv[:, 1:D], op0=AL.mult, op1=AL.add)
                nc.vector.scalar_tensor_tensor(
                    out=cv[:, 0:D - 1], in0=praw[:, 1:D], scalar=w_bc[:, 2:3],
                    in1=cv[:, 0:D - 1], op0=AL.mult, op1=AL.add)
                nc.scalar.activation(PC[:, j, :], cv[:], ACTF.Relu, bias=b_bc[:, 0:1])

            # ---- main loop over batch tiles ----
            for j in range(NT):
                if STAGE < 2:
                    t0 = work.tile([128, D], F32, tag="t1")
                    nc.scalar.copy(t0[:], X[:, j, :])
                    nc.sync.dma_start(out_d[j * 128:(j + 1) * 128, :], t0[:])
                    continue
                cos_ps = psC.tile([128, D], F32, tag="cos")
                for h in range(2):
                    hs = slice(h * 512, (h + 1) * 512)
                    for i in range(ND):
                        nc.tensor.matmul(
                            cos_ps[:, hs],
                            lhsT=XT[:, i, j * 128:(j + 1) * 128],
                            rhs=KT[:, i, hs],
                            start=(i == 0), stop=(i == ND - 1))
                cos = work.tile([128, D], F32, tag="cos_sb")
                nc.scalar.copy(cos[:], cos_ps[:])
                if STAGE < 3:
                    nc.sync.dma_start(out_d[j * 128:(j + 1) * 128, :], cos[:])
                    continue

                # entropy pieces: e = exp(1 - cos), S1 = sum e, S3 = sum cos*e
                e = work.tile([128, D], F32, tag="escr")
                s1 = small.tile([128, 1], F32, tag="s1")
                nc.scalar.activation(e[:], cos[:], ACTF.Exp, bias=1.0, scale=-1.0,
                                     accum_out=s1[:])
                mr = work.tile([128, D], F32, tag="mr")
                s3 = small.tile([128, 1], F32, tag="s3")
                nc.vector.scalar_tensor_tensor(
                    out=mr[:], in0=e[:], scalar=1.0, in1=cos[:],
                    op0=AL.mult, op1=AL.mult, accum_out=s3[:])

                # top-8 (largest cos = smallest score)
                m8 = small.tile([128, 8], F32, tag="m8")
                nc.vector.max(m8[:], cos[:])
                msum = small.tile([128, 1], F32, tag="msum")
                nc.vector.reduce_sum(msum[:], m8[:], axis=AX.X)
                nc.vector.match_replace(mr[:], m8[:], cos[:], 2.0)
                # one-hot mask of the top-8 (1.0 where replaced by 2.0)
                msk = work.tile([128, D], F32, tag="cos_sb")
                nc.scalar.activation(msk[:], mr[:], ACTF.Relu, bias=neg3[:, 0:1], scale=2.0)
                if STAGE < 4:
                    nc.sync.dma_start(out_d[j * 128:(j + 1) * 128, :], msk[:])
                    continue

                mtp = psA.tile([128, ND, 128], F32, tag="tp")
                for i in range(ND):
                    nc.tensor.transpose(mtp[:, i, :], msk[:, i * 128:(i + 1) * 128], ident[:])
                mT = work.tile([128, ND, 128], F32G, tag="mT")
                nc.scalar.copy(mT[:], mtp[:])

                g_ps = psG.tile([128, D], F32, tag="g")
                for h in range(2):
                    hs = slice(h * 512, (h + 1) * 512)
                    for i in range(ND):
                        nc.tensor.matmul(
                            g_ps[:, hs],
                            lhsT=mT[:, i, :],
                            rhs=PC[:, i, hs],
                            start=(i == 0), stop=(i == ND - 1))

                # min-max normalize rows of g (affine-invariant: /8 not needed)
                pmax = small.tile([128, 1], F32, tag="pmax")
                nc.vector.tensor_reduce(pmax[:], g_ps[:], axis=AX.X, op=AL.max)
                pmin = small.tile([128, 1], F32, tag="pmin")
                nc.vector.tensor_reduce(pmin[:], g_ps[:], axis=AX.X, op=AL.min)
                rng = small.tile([128, 1], F32, tag="rng")
                nc.vector.tensor_sub(rng[:], pmax[:], pmin[:])
                rr = small.tile([128, 1], F32, tag="rr")
                nc.vector.reciprocal(rr[:], rng[:])
                scl = small.tile([128, 1], F32, tag="scl")
                nc.vector.tensor_scalar_mul(scl[:], rr[:], 2.0)
                pms = small.tile([128, 1], F32, tag="pms")
                nc.vector.tensor_mul(pms[:], pmin[:], scl[:])
                bia = small.tile([128, 1], F32, tag="bia")
                nc.vector.tensor_scalar(bia[:], pms[:], -1.0, -1.0, AL.mult, AL.add)
                t1 = work.tile([128, D], F32, tag="t1")
                nc.scalar.activation(t1[:], g_ps[:], ACTF.Identity,
                                     bias=bia[:, 0:1], scale=scl[:, 0:1])
                if STAGE >= 5:
                    nc.gpsimd.tensor_add(t1[:], t1[:], X[:, j, :])
                nc.sync.dma_start(out_d[j * 128:(j + 1) * 128, :], t1[:])

                # per-row entropy = ln(S1) + S3/S1  (host subtracts 1)
                r1 = small.tile([128, 1], F32, tag="r1")
                nc.vector.reciprocal(r1[:], s1[:])
                ln1 = small.tile([128, 1], F32, tag="ln1")
                nc.scalar.activation(ln1[:], s1[:], ACTF.Ln)
                t3 = small.tile([128, 1], F32, tag="t3")
                nc.vector.tensor_mul(t3[:], s3[:], r1[:])
                ent = small.tile([128, 1], F32, tag="ent")
                nc.vector.tensor_add(ent[:], ln1[:], t3[:])
                nc.vector.tensor_add(acc_t[:, 0:1], acc_t[:, 0:1], ent[:])
                nc.vector.tensor_add(acc_t[:, 1:2], acc_t[:, 1:2], msum[:])

            nc.sync.dma_start(part_d[:], acc_t[:])

    nc.compile()
    _built["nc"] = nc
    return nc


def kernel(ppg, keys_pool, prompt_pool, cnn_w, cnn_b):
    from concourse.bass_utils import run_bass_kernel_spmd

    nc = _build()
    x = np.ascontiguousarray(np.asarray(ppg, np.float32).reshape(BZ, D))
    keys = np.ascontiguousarray(np.asarray(keys_pool, np.float32))
    pp = np.ascontiguousarray(np.asarray(prompt_pool, np.float32))
    w = np.ascontiguousarray(np.asarray(cnn_w, np.float32).reshape(1, 3))
    b = np.ascontiguousarray(np.asarray(cnn_b, np.float32).reshape(1, 1))

    wb = np.ascontiguousarray(np.broadcast_to(w, (128, 3)))
    bb = np.ascontiguousarray(np.broadcast_to(b, (128, 1)))
    eye = np.eye(128, dtype=np.float32)
    in_maps = [
        {"ppg": x[c * BL:(c + 1) * BL], "keys": keys, "pp": pp,
         "cw": wb, "cb": bb, "ident": eye}
        for c in range(NCORES)
    ]
    res = run_bass_kernel_spmd(nc, in_maps, list(range(NCORES)))
    outs = res.results
    prompted = np.concatenate([outs[c]["out"] for c in range(NCORES)], axis=0)
    prompted = prompted.reshape(BZ, 1, D)
    ent_sum = sum(float(outs[c]["partials"][:, 0].sum()) for c in range(NCORES))
    msum = sum(float(outs[c]["partials"][:, 1].sum()) for c in range(NCORES))
    score_mean = np.float32(1.0 - msum / (BZ * K))
    entropy = np.float32(ent_sum / BZ - 1.0)
    return prompted, score_mean, entropy
